# revision 10
# baseline (speedup 1.0000x reference)
"""Trainium2 Bass kernel for CosyVoice3 DiT attention (B=2, S=2048, H=16, hd=64, D=1024).

Sharding: tensor parallelism over heads — 2 heads per core on 8 cores.
Each core computes QKV projections for its head slice, RoPE, full attention
for its 2 heads, normalizes, then an AllGather of the per-head attention
outputs lets every core compute a disjoint 128-column slice of the output
projection. Host gather = concat + transpose (no reduction).

Layout trick: everything is computed transposed ([dim, tokens]) so the
attention matmuls need no on-chip transposes of the big S x S matrices:
  scoresT[k,q] = K @ Q^T    (lhsT = K^T slice, rhs = Q^T slice)
  outT[d,q]    = V_aug^T @ expT  with V_aug = [V | ones] giving the softmax
                 denominator for free in row 64.
Softmax skips max-subtraction (scores are O(10) for this model family, and
exp is computed in fp32 which is safe up to ~88).
"""
import sys
sys.path.insert(0, "/opt/trn_rl_repo")
from contextlib import ExitStack
import numpy as np

# NTFF profile hook shim: this image's antenv lacks axon_hooks, which
# bass_utils imports unconditionally when trace=True (and the boot-time
# installer degrades silently without it). Recreate the module and install
# the ctypes-based hook so neuron-profile traces work.
import types as _types
try:
    import antenv as _antenv
    if "antenv.axon_hooks" not in sys.modules:
        _hooks = _types.ModuleType("antenv.axon_hooks")
        _hook_box = [None]
        _hooks.set_axon_ntff_profile_hook = lambda h: _hook_box.__setitem__(0, h)
        _hooks.get_axon_ntff_profile_hook = lambda: _hook_box[0]
        sys.modules["antenv.axon_hooks"] = _hooks
        _antenv.axon_hooks = _hooks
        try:
            from trn_agent_boot.trn_boot import _ntff_profile_via_ctypes
            _hooks.set_axon_ntff_profile_hook(
                _ntff_profile_via_ctypes("/opt/axon/libaxon_pjrt.so"))
        except Exception:
            pass
except Exception:
    pass

import concourse.bass as bass
import concourse.mybir as mybir
from concourse import bacc
from concourse.tile import TileContext
from concourse.bass_interp import get_hw_module
from concourse import bass_utils
from concourse.masks import make_identity
bass_utils.upload_artifacts = lambda tmpdir: str(tmpdir)  # no S3 in container

# ── constants (hardcoded per problem spec) ────────────────────────────────
B, S, D, H, HD = 2, 2048, 1024, 16, 64
T = B * S                 # 4096 tokens
NC = 8                    # cores
HPC = H // NC             # 2 heads per core
CW = HPC * HD             # 128 cols per core
SCALE = 1.0 / np.sqrt(HD)
F32 = mybir.dt.float32
F32R = mybir.dt.float32r
AF = mybir.ActivationFunctionType

_CACHE = {}


def _build(use_mask: bool):
    nc = bacc.Bacc("TRN2", target_bir_lowering=False, debug=False, num_devices=NC)

    # inputs (per-core slices supplied by host)
    x_d = nc.dram_tensor("x", [T, D], F32, kind="ExternalInput")
    wq_d = nc.dram_tensor("wq", [D, CW], F32R, kind="ExternalInput")
    wk_d = nc.dram_tensor("wk", [D, CW], F32R, kind="ExternalInput")
    wv_d = nc.dram_tensor("wv", [D, CW], F32R, kind="ExternalInput")
    wo_d = nc.dram_tensor("wo", [D, CW], F32R, kind="ExternalInput")
    bq_d = nc.dram_tensor("bq", [CW, 1], F32, kind="ExternalInput")
    bk_d = nc.dram_tensor("bk", [CW, 1], F32, kind="ExternalInput")
    bv_d = nc.dram_tensor("bv", [CW, 1], F32, kind="ExternalInput")
    bo_d = nc.dram_tensor("bo", [CW, 1], F32, kind="ExternalInput")
    cos_d = nc.dram_tensor("cost", [CW, T], F32, kind="ExternalInput")
    sin_d = nc.dram_tensor("sint", [CW, T], F32, kind="ExternalInput")   # sign-folded
    psw_d = nc.dram_tensor("pswap", [128, 128], F32R, kind="ExternalInput")
    if use_mask:
        mt_d = nc.dram_tensor("maskt", [S, S], F32, kind="ExternalInput")

    yT_d = nc.dram_tensor("yT", [CW, T], F32, kind="ExternalOutput")

    ag_in = nc.dram_tensor("ag_in", [CW, T], F32)
    ag_out = nc.dram_tensor("ag_out", [NC * CW, T], F32, addr_space="Shared")

    NCHUNK = 8            # token chunks of 512 for projections
    CH = T // NCHUNK      # 512
    KT = S // 128         # 16 k-tiles per batch
    QC = 2                # q chunks per batch
    QW = S // QC          # 1024

    with TileContext(nc) as tc:
        with tc.tile_pool(name="persist", bufs=1) as persist, \
             tc.tile_pool(name="outp", bufs=2) as outp:

            # ── persistent tiles ────────────────────────────────────────
            ident = persist.tile([128, 128], F32, name="ident")
            make_identity(nc, ident)
            psw = persist.tile([128, 128], F32R, name="psw")
            nc.sync.dma_start(out=psw, in_=psw_d[:, :])
            wo = persist.tile([128, D // 128, CW], F32R, name="wo_sb")
            nc.sync.dma_start(out=wo, in_=wo_d.ap().rearrange("(kc p) m -> p kc m", p=128))
            bo = persist.tile([CW, 1], F32, name="bo_sb")
            nc.sync.dma_start(out=bo, in_=bo_d[:, :])
            aoT = persist.tile([128, T], F32, name="aoT")      # normalized attn out^T

            ph_sb = ExitStack()   # SBUF pools for phases 1-3
            wpool = ph_sb.enter_context(tc.tile_pool(name="weights", bufs=1))
            att_pool = ph_sb.enter_context(tc.tile_pool(name="attp", bufs=1))
            xload = ph_sb.enter_context(tc.tile_pool(name="xload", bufs=2))
            xtpool = ph_sb.enter_context(tc.tile_pool(name="xtp", bufs=10))
            chunks = ph_sb.enter_context(tc.tile_pool(name="chunks", bufs=2))
            expp = ph_sb.enter_context(tc.tile_pool(name="expp", bufs=3))

            wq = wpool.tile([128, D // 128, CW], F32R, name="wq_sb")
            wk = wpool.tile([128, D // 128, CW], F32R, name="wk_sb")
            wv = wpool.tile([128, D // 128, CW], F32R, name="wv_sb")
            for wt, wdr in ((wq, wq_d), (wk, wk_d), (wv, wv_d)):
                nc.sync.dma_start(out=wt, in_=wdr.ap().rearrange("(kc p) m -> p kc m", p=128))
            bq = wpool.tile([CW, 1], F32, name="bq_sb")
            bk = wpool.tile([CW, 1], F32, name="bk_sb")
            bv0 = wpool.tile([HD, 1], F32, name="bv0_sb")
            bv1 = wpool.tile([HD, 1], F32, name="bv1_sb")
            nc.sync.dma_start(out=bq, in_=bq_d[:, :])
            nc.sync.dma_start(out=bk, in_=bk_d[:, :])
            nc.sync.dma_start(out=bv0, in_=bv_d[0:HD, :])
            nc.sync.dma_start(out=bv1, in_=bv_d[HD:CW, :])

            qtr = att_pool.tile([128, T], F32R, name="qtr")    # rope'd Q^T
            ktr = att_pool.tile([128, T], F32R, name="ktr")    # rope'd K^T
            # V natural per k-tile: [128 tok, 2*(64+1)] with ones cols
            vnat = [att_pool.tile([128, 2 * (HD + 1)], F32R, name=f"vnat{i}")
                    for i in range(T // 128)]

            # ── phase 1+2: x transpose, QKV projections, rope ───────────
            ps12 = ExitStack()
            ps_a = ps12.enter_context(tc.tile_pool(name="ps_a", bufs=2, space="PSUM"))
            ps_qkv = ps12.enter_context(tc.tile_pool(name="ps_qkv", bufs=1, space="PSUM"))

            for n in range(NCHUNK):
                tcol = n * CH
                xts = [xtpool.tile([128, CH], F32R, name=f"xt{n}_{dc}", tag="xt")
                       for dc in range(D // 128)]
                for tt in range(CH // 128):
                    xn = xload.tile([128, D], F32, name=f"xn{n}_{tt}", tag="xn")
                    nc.sync.dma_start(out=xn, in_=x_d[tcol + 128 * tt: tcol + 128 * (tt + 1), :])
                    for dc in range(D // 128):
                        xp = ps_a.tile([128, 128], F32, name=f"xp{n}{tt}{dc}", tag="tp")
                        nc.tensor.transpose(xp[:, :], xn[:, 128 * dc:128 * (dc + 1)], ident)
                        nc.any.tensor_copy(xts[dc][:, 128 * tt:128 * (tt + 1)], xp[:, :])

                cos_c = chunks.tile([128, CH], F32, name=f"cos{n}", tag="cosc")
                sin_c = chunks.tile([128, CH], F32, name=f"sin{n}", tag="sinc")
                nc.sync.dma_start(out=cos_c, in_=cos_d[:, tcol:tcol + CH])
                nc.sync.dma_start(out=sin_c, in_=sin_d[:, tcol:tcol + CH])

                for name, wt, dst in (("q", wq, qtr), ("k", wk, ktr), ("v", wv, None)):
                    pp = ps_qkv.tile([128, CH], F32, name=f"{name}ps{n}", tag=f"{name}ps")
                    for dc in range(D // 128):
                        nc.tensor.matmul(pp[:, :], wt[:, dc, :], xts[dc][:, :],
                                         start=(dc == 0), stop=(dc == D // 128 - 1))
                    if name == "v":
                        # per-head natural V via PE transpose (+bias on copy out)
                        for h in range(HPC):
                            vth = chunks.tile([HD + 1, CH], F32, name=f"vt{n}{h}", tag="vth")
                            nc.scalar.activation(vth[0:HD, :], pp[HD * h:HD * (h + 1), :],
                                                 AF.Identity, bias=(bv0 if h == 0 else bv1))
                            nc.vector.memset(vth[HD:HD + 1, :], 1.0)
                            for ktl in range(CH // 128):
                                vp = ps_a.tile([128, HD + 1], F32, name=f"vp{n}{h}{ktl}", tag="tp")
                                nc.tensor.transpose(vp[:, :], vth[:, 128 * ktl:128 * (ktl + 1)],
                                                    ident[0:HD + 1, 0:HD + 1])
                                kt_glob = (tcol + 128 * ktl) // 128
                                nc.any.tensor_copy(
                                    vnat[kt_glob][:, 65 * h:65 * h + HD + 1], vp[:, :])
                    else:
                        # bias + rope: dst_chunk = (p+b)*cos + swap(p+b)*sin_signed
                        qb = chunks.tile([128, CH], F32R, name=f"{name}b{n}", tag="qb")
                        nc.scalar.activation(qb[:, :], pp[:, :], AF.Identity,
                                             bias=(bq if name == "q" else bk))
                        sw = ps_a.tile([128, CH], F32, name=f"{name}sw{n}", tag="sw")
                        for j in range(CH // 512):
                            nc.tensor.matmul(sw[:, 512 * j:512 * (j + 1)], psw,
                                             qb[:, 512 * j:512 * (j + 1)],
                                             start=True, stop=True)
                        t1 = chunks.tile([128, CH], F32, name=f"{name}t1{n}", tag="t1")
                        t2 = chunks.tile([128, CH], F32, name=f"{name}t2{n}", tag="t2")
                        nc.vector.tensor_mul(t1[:, :], qb[:, :], cos_c[:, :])
                        nc.vector.tensor_mul(t2[:, :], sw[:, :], sin_c[:, :])
                        nc.vector.tensor_add(dst[:, tcol:tcol + CH], t1[:, :], t2[:, :])

            ps12.close()

            # ── phase 3: attention per (batch, head) ────────────────────
            ps_att = tc.alloc_tile_pool(name="ps_att", bufs=2, space="PSUM")
            for b in range(B):
                toff = b * S
                for h in range(HPC):
                    po = HD * h
                    for qc in range(QC):
                        qcols = slice(toff + QW * qc, toff + QW * (qc + 1))
                        ot = ps_att.tile([HD + 1, QW], F32, name=f"ot{b}{h}{qc}", tag="ot")
                        for kt in range(KT):
                            krows = slice(toff + 128 * kt, toff + 128 * (kt + 1))
                            sc = ps_att.tile([128, QW], F32, name=f"sc{b}{h}{qc}{kt}", tag="sc")
                            for j in range(QW // 512):
                                nc.tensor.matmul(
                                    sc[:, 512 * j:512 * (j + 1)],
                                    ktr[po:po + HD, krows],
                                    qtr[po:po + HD, qcols][:, 512 * j:512 * (j + 1)],
                                    start=True, stop=True)
                            if use_mask:
                                mtile = expp.tile([128, QW], F32, name=f"mt{b}{h}{qc}{kt}", tag="mt")
                                nc.sync.dma_start(
                                    out=mtile,
                                    in_=mt_d[128 * kt:128 * (kt + 1), QW * qc:QW * (qc + 1)])
                                nc.vector.tensor_scalar_mul(sc[:, :], sc[:, :], SCALE)
                                nc.vector.tensor_add(sc[:, :], sc[:, :], mtile[:, :])
                            ex = expp.tile([128, QW], F32R, name=f"ex{b}{h}{qc}{kt}", tag="ex")
                            nc.scalar.activation(ex[:, :], sc[:, :], AF.Exp,
                                                 scale=(1.0 if use_mask else SCALE))
                            kt_glob = (toff + 128 * kt) // 128
                            for j in range(QW // 512):
                                nc.tensor.matmul(
                                    ot[:, 512 * j:512 * (j + 1)],
                                    vnat[kt_glob][:, 65 * h:65 * h + HD + 1],
                                    ex[:, 512 * j:512 * (j + 1)],
                                    start=(kt == 0), stop=(kt == KT - 1))
                        # normalize: rows 0..63 divided by row 64
                        rec = outp.tile([1, QW], F32, name=f"rec{b}{h}{qc}", tag="rec")
                        nc.vector.reciprocal(rec[:, :], ot[HD:HD + 1, :])
                        bcast = outp.tile([HD, QW], F32, name=f"bc{b}{h}{qc}", tag="bc")
                        nc.gpsimd.partition_broadcast(bcast[:, :], rec[:, :])
                        nc.vector.tensor_mul(aoT[po:po + HD, qcols], ot[0:HD, :], bcast[:, :])
            ps_att.release()

            # ── phase 4: AllGather + output projection slice ────────────
            nc.sync.dma_start(out=ag_in[:, :], in_=aoT[:, :])
            ph_sb.close()
            agp = tc.alloc_tile_pool(name="agp", bufs=10)
            ps_y = tc.alloc_tile_pool(name="ps_y", bufs=2, space="PSUM")

            nc.gpsimd.collective_compute(
                "AllGather", mybir.AluOpType.bypass,
                replica_groups=[list(range(NC))],
                ins=[ag_in.ap()], outs=[ag_out.ap()])
            for n in range(NCHUNK):
                tcol = n * CH
                ats = []
                for dc in range(D // 128):
                    at = agp.tile([128, CH], F32R, name=f"at{n}{dc}", tag="at")
                    nc.sync.dma_start(
                        out=at, in_=ag_out[128 * dc:128 * (dc + 1), tcol:tcol + CH].bitcast(F32R))
                    ats.append(at)
                yp = ps_y.tile([128, CH], F32, name=f"yp{n}", tag="yp")
                for dc in range(D // 128):
                    nc.tensor.matmul(yp[:, :], wo[:, dc, :], ats[dc][:, :],
                                     start=(dc == 0), stop=(dc == D // 128 - 1))
                yo = outp.tile([128, CH], F32, name=f"yo{n}", tag="yo")
                nc.scalar.activation(yo[:, :], yp[:, :], AF.Identity, bias=bo)
                nc.sync.dma_start(out=yT_d[:, tcol:tcol + CH], in_=yo)
            agp.release()
            ps_y.release()

    nc.compile()
    nc.m = get_hw_module(nc.m)
    return nc


def _get_nc(use_mask: bool):
    key = ("nc", use_mask)
    if key not in _CACHE:
        _CACHE[key] = _build(use_mask)
    return _CACHE[key]


def kernel(x, rope, mask, Wq, bq, Wk, bk, Wv, bv, Wo, bo, _trace=False):
    x = np.ascontiguousarray(np.asarray(x, dtype=np.float32))
    rope = np.asarray(rope, dtype=np.float32)
    mask = np.asarray(mask, dtype=np.float32)
    use_mask = bool(np.any(mask))

    x2d = x.reshape(T, D)
    cos = rope[0, 0, :, 0, :]                      # [S, 64]
    sin = rope[1, 0, :, 0, :]
    sgn = np.where(np.arange(HD) % 2 == 0, -1.0, 1.0).astype(np.float32)[:, None]
    cosT = np.ascontiguousarray(np.tile(cos.T, (HPC, B)))          # [128, T]
    sinT = np.ascontiguousarray(np.tile(sin.T * sgn, (HPC, B)))    # [128, T]
    psw = np.zeros((128, 128), dtype=np.float32)
    idx = np.arange(128)
    psw[idx ^ 1, idx] = 1.0

    nc = _get_nc(use_mask)
    in_maps = []
    for c in range(NC):
        cs = slice(CW * c, CW * (c + 1))
        m = dict(
            x=x2d,
            wq=np.ascontiguousarray(Wq[:, cs]), bq=np.ascontiguousarray(bq[cs]).reshape(CW, 1),
            wk=np.ascontiguousarray(Wk[:, cs]), bk=np.ascontiguousarray(bk[cs]).reshape(CW, 1),
            wv=np.ascontiguousarray(Wv[:, cs]), bv=np.ascontiguousarray(bv[cs]).reshape(CW, 1),
            wo=np.ascontiguousarray(Wo[:, cs]), bo=np.ascontiguousarray(bo[cs]).reshape(CW, 1),
            cost=cosT, sint=sinT, pswap=psw,
        )
        if use_mask:
            m["maskt"] = np.ascontiguousarray(mask[0, 0].T)
        in_maps.append({k: np.asarray(v, dtype=np.float32) for k, v in m.items()})

    res = bass_utils.run_bass_kernel_spmd(
        nc, in_maps, core_ids=list(range(NC)), trace=_trace)
    yT = np.concatenate([res.results[c]["yT"] for c in range(NC)], axis=0)  # [1024, T]
    out = np.ascontiguousarray(yT.T).reshape(B, S, D).astype(np.float32)
    if _trace:
        return out, res
    return out


# revision 13
# speedup vs baseline: 1.5030x; 1.5030x over previous
"""Trainium2 Bass kernel for CosyVoice3 DiT attention (B=2, S=2048, H=16, hd=64, D=1024).

Sharding: tensor parallelism over heads — 2 heads per core on 8 cores.
Each core computes QKV projections for its head slice, RoPE, full attention
for its 2 heads, then its heads' contribution to the output projection
(row-parallel). The host gather sums the 8 partial outputs (the standard
row-parallel TP reduction) and adds the output bias.

Layout trick: everything is computed transposed ([dim, tokens]) so the
attention matmuls need no on-chip transposes of the big S x S matrices:
  scoresT[k,q] = K @ Q^T    (lhsT = K^T slice, rhs = Q^T slice)
  outT[d,q]    = V_aug^T @ expT  with V_aug = [V | ones] giving the softmax
                 denominator for free in row 64.
Softmax skips max-subtraction (scores are O(10) for this model family, and
exp is computed in fp32 which is safe up to ~88).

The emission order interleaves batch-0 attention with the tail of the
QKV-projection phase so the PE never idles long enough for the HAM clock
gate to re-throttle it to 1.2 GHz.
"""
import sys
sys.path.insert(0, "/opt/trn_rl_repo")
from contextlib import ExitStack
import numpy as np

# NTFF profile hook shim: this image's antenv lacks axon_hooks, which
# bass_utils imports unconditionally when trace=True (and the boot-time
# installer degrades silently without it). Recreate the module and install
# the ctypes-based hook so neuron-profile traces work.
import types as _types
try:
    import antenv as _antenv
    if "antenv.axon_hooks" not in sys.modules:
        _hooks = _types.ModuleType("antenv.axon_hooks")
        _hook_box = [None]
        _hooks.set_axon_ntff_profile_hook = lambda h: _hook_box.__setitem__(0, h)
        _hooks.get_axon_ntff_profile_hook = lambda: _hook_box[0]
        sys.modules["antenv.axon_hooks"] = _hooks
        _antenv.axon_hooks = _hooks
        try:
            from trn_agent_boot.trn_boot import _ntff_profile_via_ctypes
            _hooks.set_axon_ntff_profile_hook(
                _ntff_profile_via_ctypes("/opt/axon/libaxon_pjrt.so"))
        except Exception:
            pass
except Exception:
    pass

import concourse.bass as bass
import concourse.mybir as mybir
from concourse import bacc
from concourse.tile import TileContext
from concourse.bass_interp import get_hw_module
from concourse import bass_utils
from concourse.masks import make_identity
bass_utils.upload_artifacts = lambda tmpdir: str(tmpdir)  # no S3 in container

# ── constants (hardcoded per problem spec) ────────────────────────────────
B, S, D, H, HD = 2, 2048, 1024, 16, 64
T = B * S                 # 4096 tokens
NC = 8                    # cores
HPC = H // NC             # 2 heads per core
CW = HPC * HD             # 128 rows/cols per core
SCALE = 1.0 / np.sqrt(HD)
F32 = mybir.dt.float32
F32R = mybir.dt.float32r
AF = mybir.ActivationFunctionType

_CACHE = {}


def _build(use_mask: bool):
    nc = bacc.Bacc("TRN2", target_bir_lowering=False, debug=False, num_devices=NC)

    # inputs (per-core slices supplied by host)
    x_d = nc.dram_tensor("x", [T, D], F32, kind="ExternalInput")
    wq_d = nc.dram_tensor("wq", [D, CW], F32R, kind="ExternalInput")
    wk_d = nc.dram_tensor("wk", [D, CW], F32R, kind="ExternalInput")
    wv_d = nc.dram_tensor("wv", [D, CW], F32R, kind="ExternalInput")
    # wo: the CW rows of Wo owned by this core's heads -> [CW, D]
    wo_d = nc.dram_tensor("wo", [CW, D], F32R, kind="ExternalInput")
    bq_d = nc.dram_tensor("bq", [CW, 1], F32, kind="ExternalInput")
    bk_d = nc.dram_tensor("bk", [CW, 1], F32, kind="ExternalInput")
    bv_d = nc.dram_tensor("bv", [CW, 1], F32, kind="ExternalInput")
    cos_d = nc.dram_tensor("cost", [CW, T], F32, kind="ExternalInput")
    sin_d = nc.dram_tensor("sint", [CW, T], F32, kind="ExternalInput")   # sign-folded
    psw_d = nc.dram_tensor("pswap", [128, 128], F32R, kind="ExternalInput")
    if use_mask:
        mt_d = nc.dram_tensor("maskt", [S, S], F32, kind="ExternalInput")

    # partial output, transposed: ypT[n, t] = sum over this core's head dims
    ypT_d = nc.dram_tensor("ypT", [D, T], F32, kind="ExternalOutput")

    NCHUNK = 8            # token chunks of 512 for projections
    CH = T // NCHUNK      # 512
    KT = S // 128         # 16 k-tiles per batch
    QW = 512              # q chunk width
    QC = S // QW          # 4 q chunks per batch

    with TileContext(nc) as tc:
        with tc.tile_pool(name="persist", bufs=1) as persist, \
             tc.tile_pool(name="wpool", bufs=1) as wpool, \
             tc.tile_pool(name="xload", bufs=6) as xload, \
             tc.tile_pool(name="xtp", bufs=10) as xtpool, \
             tc.tile_pool(name="chunks", bufs=2) as chunks, \
             tc.tile_pool(name="expp", bufs=4) as expp, \
             tc.tile_pool(name="outp", bufs=3) as outp, \
             tc.tile_pool(name="ps_tp", bufs=2, space="PSUM") as ps_tp, \
             tc.tile_pool(name="ps_proj", bufs=2, space="PSUM") as ps_proj, \
             tc.tile_pool(name="ps_sc", bufs=2, space="PSUM") as ps_sc, \
             tc.tile_pool(name="ps_ot", bufs=2, space="PSUM") as ps_ot:

            # ── persistent tiles ────────────────────────────────────────
            ident = persist.tile([128, 128], F32, name="ident")
            make_identity(nc, ident)
            psw = persist.tile([128, 128], F32R, name="psw")
            nc.sync.dma_start(out=psw, in_=psw_d[:, :])
            wq = wpool.tile([128, D // 128, CW], F32R, name="wq_sb")
            wk = wpool.tile([128, D // 128, CW], F32R, name="wk_sb")
            wv = wpool.tile([128, D // 128, CW], F32R, name="wv_sb")
            for wt, wdr in ((wq, wq_d), (wk, wk_d), (wv, wv_d)):
                nc.sync.dma_start(out=wt, in_=wdr.ap().rearrange("(kc p) m -> p kc m", p=128))
            # wo rows for this core: [CW, D] -> lhsT chunks [CW, 128] per out-col group
            wo = wpool.tile([CW, D // 128, 128], F32R, name="wo_sb")
            nc.sync.dma_start(out=wo, in_=wo_d.ap().rearrange("p (mc m) -> p mc m", m=128))
            bq = wpool.tile([CW, 1], F32, name="bq_sb")
            bk = wpool.tile([CW, 1], F32, name="bk_sb")
            bv0 = wpool.tile([HD, 1], F32, name="bv0_sb")
            bv1 = wpool.tile([HD, 1], F32, name="bv1_sb")
            nc.sync.dma_start(out=bq, in_=bq_d[:, :])
            nc.sync.dma_start(out=bk, in_=bk_d[:, :])
            nc.sync.dma_start(out=bv0, in_=bv_d[0:HD, :])
            nc.sync.dma_start(out=bv1, in_=bv_d[HD:CW, :])

            qtr = persist.tile([128, T], F32R, name="qtr")    # rope'd Q^T
            ktr = persist.tile([128, T], F32R, name="ktr")    # rope'd K^T
            aoT = persist.tile([128, T], F32R, name="aoT")    # normalized attn out^T
            # V natural per k-tile: [128 tok, 2*(64+1)] with ones cols
            vnat = [persist.tile([128, 2 * (HD + 1)], F32R, name=f"vnat{i}")
                    for i in range(T // 128)]

            # ── phase 1: per token-chunk: transpose x, QKV proj, rope ───
            def emit_chunk(n):
                tcol = n * CH
                xts = [xtpool.tile([128, CH], F32R, name=f"xt{n}_{dc}", tag="xt")
                       for dc in range(D // 128)]
                xns = []
                for tt in range(CH // 128):
                    xn = xload.tile([128, D], F32, name=f"xn{n}_{tt}", tag="xn")
                    nc.sync.dma_start(out=xn, in_=x_d[tcol + 128 * tt: tcol + 128 * (tt + 1), :])
                    xns.append(xn)
                for dc in range(D // 128):
                    # pack the 4 token-block transposes of one d-block into one
                    # psum bank, one copy out
                    xp = ps_tp.tile([128, CH], F32, name=f"xp{n}{dc}", tag="tp")
                    for tt in range(CH // 128):
                        nc.tensor.transpose(xp[:, 128 * tt:128 * (tt + 1)],
                                            xns[tt][:, 128 * dc:128 * (dc + 1)], ident)
                    nc.any.tensor_copy(xts[dc][:, :], xp[:, :])

                cos_c = chunks.tile([128, CH], F32, name=f"cos{n}", tag="cosc")
                sin_c = chunks.tile([128, CH], F32, name=f"sin{n}", tag="sinc")
                nc.sync.dma_start(out=cos_c, in_=cos_d[:, tcol:tcol + CH])
                nc.sync.dma_start(out=sin_c, in_=sin_d[:, tcol:tcol + CH])

                for name, wt, dst in (("q", wq, qtr), ("k", wk, ktr), ("v", wv, None)):
                    pp = ps_proj.tile([128, CH], F32, name=f"{name}ps{n}", tag="proj")
                    for dc in range(D // 128):
                        nc.tensor.matmul(pp[:, :], wt[:, dc, :], xts[dc][:, :],
                                         start=(dc == 0), stop=(dc == D // 128 - 1))
                    if name == "v":
                        # per-head natural V via PE transpose; ones row becomes
                        # the denominator column after transpose
                        for h in range(HPC):
                            vth = chunks.tile([HD + 1, CH], F32, name=f"vt{n}{h}", tag="vth")
                            nc.scalar.activation(vth[0:HD, :], pp[HD * h:HD * (h + 1), :],
                                                 AF.Identity, bias=(bv0 if h == 0 else bv1))
                            nc.vector.memset(vth[HD:HD + 1, :], 1.0)
                            for ktl in range(CH // 128):
                                vp = ps_tp.tile([128, HD + 1], F32, name=f"vp{n}{h}{ktl}", tag="tp")
                                nc.tensor.transpose(vp[:, :], vth[:, 128 * ktl:128 * (ktl + 1)],
                                                    ident[0:HD + 1, 0:HD + 1])
                                kt_glob = (tcol + 128 * ktl) // 128
                                nc.any.tensor_copy(
                                    vnat[kt_glob][:, 65 * h:65 * h + HD + 1], vp[:, :])
                    else:
                        # bias + rope: dst_chunk = (p+b)*cos + swap(p+b)*sin_signed
                        qb = chunks.tile([128, CH], F32R, name=f"{name}b{n}", tag="qb")
                        nc.scalar.activation(qb[:, :], pp[:, :], AF.Identity,
                                             bias=(bq if name == "q" else bk))
                        sw = ps_tp.tile([128, CH], F32, name=f"{name}sw{n}", tag="tp")
                        for j in range(CH // 512):
                            nc.tensor.matmul(sw[:, 512 * j:512 * (j + 1)], psw,
                                             qb[:, 512 * j:512 * (j + 1)],
                                             start=True, stop=True)
                        t1 = chunks.tile([128, CH], F32, name=f"{name}t1{n}", tag="t1")
                        t2 = chunks.tile([128, CH], F32, name=f"{name}t2{n}", tag="t2")
                        nc.vector.tensor_mul(t1[:, :], qb[:, :], cos_c[:, :])
                        nc.vector.tensor_mul(t2[:, :], sw[:, :], sin_c[:, :])
                        nc.vector.tensor_add(dst[:, tcol:tcol + CH], t1[:, :], t2[:, :])

            # ── phase 3: attention for one (batch, head, q-chunk) ───────
            def emit_att(b, h, qc):
                toff = b * S
                po = HD * h
                qcols = slice(toff + QW * qc, toff + QW * (qc + 1))
                ot = ps_ot.tile([HD + 1, QW], F32, name=f"ot{b}{h}{qc}", tag="ot")
                for kt in range(KT):
                    krows = slice(toff + 128 * kt, toff + 128 * (kt + 1))
                    sc = ps_sc.tile([128, QW], F32, name=f"sc{b}{h}{qc}{kt}", tag="sc")
                    nc.tensor.matmul(sc[:, :], ktr[po:po + HD, krows],
                                     qtr[po:po + HD, qcols], start=True, stop=True)
                    if use_mask:
                        mtile = expp.tile([128, QW], F32, name=f"mt{b}{h}{qc}{kt}", tag="mt")
                        nc.sync.dma_start(
                            out=mtile,
                            in_=mt_d[128 * kt:128 * (kt + 1), QW * qc:QW * (qc + 1)])
                        nc.vector.tensor_scalar_mul(sc[:, :], sc[:, :], SCALE)
                        nc.vector.tensor_add(sc[:, :], sc[:, :], mtile[:, :])
                    ex = expp.tile([128, QW], F32R, name=f"ex{b}{h}{qc}{kt}", tag="ex")
                    nc.scalar.activation(ex[:, :], sc[:, :], AF.Exp,
                                         scale=(1.0 if use_mask else SCALE))
                    kt_glob = (toff + 128 * kt) // 128
                    nc.tensor.matmul(ot[:, :], vnat[kt_glob][:, 65 * h:65 * h + HD + 1],
                                     ex[:, :], start=(kt == 0), stop=(kt == KT - 1))
                # normalize: rows 0..63 divided by row 64
                rec = outp.tile([1, QW], F32, name=f"rec{b}{h}{qc}", tag="rec")
                nc.vector.reciprocal(rec[:, :], ot[HD:HD + 1, :])
                bcast = outp.tile([HD, QW], F32, name=f"bc{b}{h}{qc}", tag="bc")
                nc.gpsimd.partition_broadcast(bcast[:, :], rec[:, :])
                nc.vector.tensor_mul(aoT[po:po + HD, qcols], ot[0:HD, :], bcast[:, :])

            # ── phase 4: partial output projection for one (batch,qchunk)
            def emit_oproj(b, qc):
                toff = b * S
                qcols = slice(toff + QW * qc, toff + QW * (qc + 1))
                for mo in range(D // 128):
                    yp = ps_proj.tile([128, QW], F32, name=f"yp{b}{qc}{mo}", tag="proj")
                    nc.tensor.matmul(yp[:, :], wo[:, mo, :], aoT[:, qcols],
                                     start=True, stop=True)
                    yo = outp.tile([128, QW], F32, name=f"yo{b}{qc}{mo}", tag="yo")
                    nc.any.tensor_copy(yo[:, :], yp[:, :])
                    nc.sync.dma_start(out=ypT_d[128 * mo:128 * (mo + 1), qcols], in_=yo)

            # ── emission order ──────────────────────────────────────────
            import os as _os
            if _os.environ.get("KORDER", "interleave") == "seq":
                for n in range(NCHUNK):
                    emit_chunk(n)
                for b in range(B):
                    for h in range(HPC):
                        for qc in range(QC):
                            emit_att(b, h, qc)
                    for qc in range(QC):
                        emit_oproj(b, qc)
            else:
                for n in range(4):
                    emit_chunk(n)
                for qc in range(QC):
                    emit_att(0, 0, qc)
                    emit_chunk(4 + qc)
                for qc in range(QC):
                    emit_att(0, 1, qc)
                    emit_oproj(0, qc)
                for qc in range(QC):
                    emit_att(1, 0, qc)
                for qc in range(QC):
                    emit_att(1, 1, qc)
                    emit_oproj(1, qc)

    nc.compile()
    nc.m = get_hw_module(nc.m)
    return nc


def _get_nc(use_mask: bool):
    key = ("nc", use_mask)
    if key not in _CACHE:
        _CACHE[key] = _build(use_mask)
    return _CACHE[key]


def kernel(x, rope, mask, Wq, bq, Wk, bk, Wv, bv, Wo, bo, _trace=False):
    x = np.ascontiguousarray(np.asarray(x, dtype=np.float32))
    rope = np.asarray(rope, dtype=np.float32)
    mask = np.asarray(mask, dtype=np.float32)
    use_mask = bool(np.any(mask))

    x2d = x.reshape(T, D)
    cos = rope[0, 0, :, 0, :]                      # [S, 64]
    sin = rope[1, 0, :, 0, :]
    sgn = np.where(np.arange(HD) % 2 == 0, -1.0, 1.0).astype(np.float32)[:, None]
    cosT = np.ascontiguousarray(np.tile(cos.T, (HPC, B)))          # [128, T]
    sinT = np.ascontiguousarray(np.tile(sin.T * sgn, (HPC, B)))    # [128, T]
    psw = np.zeros((128, 128), dtype=np.float32)
    idx = np.arange(128)
    psw[idx ^ 1, idx] = 1.0

    nc = _get_nc(use_mask)
    in_maps = []
    for c in range(NC):
        cs = slice(CW * c, CW * (c + 1))
        m = dict(
            x=x2d,
            wq=np.ascontiguousarray(Wq[:, cs]), bq=np.ascontiguousarray(bq[cs]).reshape(CW, 1),
            wk=np.ascontiguousarray(Wk[:, cs]), bk=np.ascontiguousarray(bk[cs]).reshape(CW, 1),
            wv=np.ascontiguousarray(Wv[:, cs]), bv=np.ascontiguousarray(bv[cs]).reshape(CW, 1),
            wo=np.ascontiguousarray(Wo[cs, :]),
            cost=cosT, sint=sinT, pswap=psw,
        )
        if use_mask:
            m["maskt"] = np.ascontiguousarray(mask[0, 0].T)
        in_maps.append({k: np.asarray(v, dtype=np.float32) for k, v in m.items()})

    res = bass_utils.run_bass_kernel_spmd(
        nc, in_maps, core_ids=list(range(NC)), trace=_trace)
    # row-parallel unshard: sum the per-core partial projections, add bias
    ypT = res.results[0]["ypT"].astype(np.float32)
    for c in range(1, NC):
        ypT = ypT + res.results[c]["ypT"]
    out = (ypT.T + np.asarray(bo, dtype=np.float32)).reshape(B, S, D).astype(np.float32)
    out = np.ascontiguousarray(out)
    if _trace:
        return out, res
    return out


# revision 14
# speedup vs baseline: 1.6764x; 1.1154x over previous
"""Trainium2 Bass kernel for CosyVoice3 DiT attention (B=2, S=2048, H=16, hd=64, D=1024).

Sharding: tensor parallelism over heads — 2 heads per core on 8 cores.
Each core computes QKV projections for its head slice, RoPE, full attention
for its 2 heads, then its heads' contribution to the output projection
(row-parallel). The host gather sums the 8 partial outputs (the standard
row-parallel TP reduction) and adds the output bias.

Layout trick: everything is computed transposed ([dim, tokens]) so the
attention matmuls need no on-chip transposes of the big S x S matrices:
  scoresT[k,q] = K @ Q^T    (lhsT = K^T slice, rhs = Q^T slice)
  outT[d,q]    = V_aug^T @ expT  with V_aug = [V | ones] giving the softmax
                 denominator for free in row 64.
Softmax skips max-subtraction (scores are O(10) for this model family, and
exp is computed in fp32 which is safe up to ~88).

The emission order interleaves batch-0 attention with the tail of the
QKV-projection phase so the PE never idles long enough for the HAM clock
gate to re-throttle it to 1.2 GHz.
"""
import sys
sys.path.insert(0, "/opt/trn_rl_repo")
from contextlib import ExitStack
import numpy as np

# NTFF profile hook shim: this image's antenv lacks axon_hooks, which
# bass_utils imports unconditionally when trace=True (and the boot-time
# installer degrades silently without it). Recreate the module and install
# the ctypes-based hook so neuron-profile traces work.
import types as _types
try:
    import antenv as _antenv
    if "antenv.axon_hooks" not in sys.modules:
        _hooks = _types.ModuleType("antenv.axon_hooks")
        _hook_box = [None]
        _hooks.set_axon_ntff_profile_hook = lambda h: _hook_box.__setitem__(0, h)
        _hooks.get_axon_ntff_profile_hook = lambda: _hook_box[0]
        sys.modules["antenv.axon_hooks"] = _hooks
        _antenv.axon_hooks = _hooks
        try:
            from trn_agent_boot.trn_boot import _ntff_profile_via_ctypes
            _hooks.set_axon_ntff_profile_hook(
                _ntff_profile_via_ctypes("/opt/axon/libaxon_pjrt.so"))
        except Exception:
            pass
except Exception:
    pass

import concourse.bass as bass
import concourse.mybir as mybir
from concourse import bacc
from concourse.tile import TileContext
from concourse.bass_interp import get_hw_module
from concourse import bass_utils
from concourse.masks import make_identity
bass_utils.upload_artifacts = lambda tmpdir: str(tmpdir)  # no S3 in container

# ── constants (hardcoded per problem spec) ────────────────────────────────
B, S, D, H, HD = 2, 2048, 1024, 16, 64
T = B * S                 # 4096 tokens
NC = 8                    # cores
HPC = H // NC             # 2 heads per core
CW = HPC * HD             # 128 rows/cols per core
SCALE = 1.0 / np.sqrt(HD)
F32 = mybir.dt.float32
F32R = mybir.dt.float32r
BF16 = mybir.dt.bfloat16
AF = mybir.ActivationFunctionType

_CACHE = {}


def _build(use_mask: bool):
    nc = bacc.Bacc("TRN2", target_bir_lowering=False, debug=False, num_devices=NC)

    # inputs (per-core slices supplied by host)
    x_d = nc.dram_tensor("x", [T, D], F32, kind="ExternalInput")
    wq_d = nc.dram_tensor("wq", [D, CW], F32R, kind="ExternalInput")
    wk_d = nc.dram_tensor("wk", [D, CW], F32R, kind="ExternalInput")
    wv_d = nc.dram_tensor("wv", [D, CW], F32R, kind="ExternalInput")
    # wo: the CW rows of Wo owned by this core's heads -> [CW, D]
    wo_d = nc.dram_tensor("wo", [CW, D], F32R, kind="ExternalInput")
    bq_d = nc.dram_tensor("bq", [CW, 1], F32, kind="ExternalInput")
    bk_d = nc.dram_tensor("bk", [CW, 1], F32, kind="ExternalInput")
    bv_d = nc.dram_tensor("bv", [CW, 1], F32, kind="ExternalInput")
    cos_d = nc.dram_tensor("cost", [CW, T], F32, kind="ExternalInput")
    sin_d = nc.dram_tensor("sint", [CW, T], F32, kind="ExternalInput")   # sign-folded
    psw_d = nc.dram_tensor("pswap", [128, 128], F32R, kind="ExternalInput")
    if use_mask:
        mt_d = nc.dram_tensor("maskt", [S, S], F32, kind="ExternalInput")

    # partial output, transposed: ypT[n, t] = sum over this core's head dims
    ypT_d = nc.dram_tensor("ypT", [D, T], F32, kind="ExternalOutput")

    NCHUNK = 8            # token chunks of 512 for projections
    CH = T // NCHUNK      # 512
    KT = S // 128         # 16 k-tiles per batch
    QW = 512              # q chunk width
    QC = S // QW          # 4 q chunks per batch

    with TileContext(nc) as tc:
        with tc.tile_pool(name="persist", bufs=1) as persist, \
             tc.tile_pool(name="wpool", bufs=1) as wpool, \
             tc.tile_pool(name="xload", bufs=6) as xload, \
             tc.tile_pool(name="xtp", bufs=10) as xtpool, \
             tc.tile_pool(name="chunks", bufs=2) as chunks, \
             tc.tile_pool(name="expp", bufs=4) as expp, \
             tc.tile_pool(name="outp", bufs=3) as outp, \
             tc.tile_pool(name="ps_tp", bufs=2, space="PSUM") as ps_tp, \
             tc.tile_pool(name="ps_proj", bufs=2, space="PSUM") as ps_proj, \
             tc.tile_pool(name="ps_sc", bufs=2, space="PSUM") as ps_sc, \
             tc.tile_pool(name="ps_ot", bufs=2, space="PSUM") as ps_ot:

            # ── persistent tiles ────────────────────────────────────────
            ident = persist.tile([128, 128], F32, name="ident")
            make_identity(nc, ident)
            psw = persist.tile([128, 128], F32R, name="psw")
            nc.sync.dma_start(out=psw, in_=psw_d[:, :])
            wq = wpool.tile([128, D // 128, CW], F32R, name="wq_sb")
            wk = wpool.tile([128, D // 128, CW], F32R, name="wk_sb")
            wv = wpool.tile([128, D // 128, CW], F32R, name="wv_sb")
            for wt, wdr in ((wq, wq_d), (wk, wk_d), (wv, wv_d)):
                nc.sync.dma_start(out=wt, in_=wdr.ap().rearrange("(kc p) m -> p kc m", p=128))
            # wo rows for this core: [CW, D] -> lhsT chunks [CW, 128] per out-col group
            wo = wpool.tile([CW, D // 128, 128], F32R, name="wo_sb")
            nc.sync.dma_start(out=wo, in_=wo_d.ap().rearrange("p (mc m) -> p mc m", m=128))
            bq = wpool.tile([CW, 1], F32, name="bq_sb")
            bk = wpool.tile([CW, 1], F32, name="bk_sb")
            bv0 = wpool.tile([HD, 1], F32, name="bv0_sb")
            bv1 = wpool.tile([HD, 1], F32, name="bv1_sb")
            nc.sync.dma_start(out=bq, in_=bq_d[:, :])
            nc.sync.dma_start(out=bk, in_=bk_d[:, :])
            nc.sync.dma_start(out=bv0, in_=bv_d[0:HD, :])
            nc.sync.dma_start(out=bv1, in_=bv_d[HD:CW, :])

            qtr = persist.tile([128, T], BF16, name="qtr")    # rope'd Q^T
            ktr = persist.tile([128, T], BF16, name="ktr")    # rope'd K^T
            aoT = persist.tile([128, T], F32R, name="aoT")    # normalized attn out^T
            # V natural per k-tile: [128 tok, 2*(64+1)] with ones cols
            vnat = [persist.tile([128, 2 * (HD + 1)], BF16, name=f"vnat{i}")
                    for i in range(T // 128)]

            # ── phase 1: per token-chunk: transpose x, QKV proj, rope ───
            def emit_chunk(n):
                tcol = n * CH
                xts = [xtpool.tile([128, CH], F32R, name=f"xt{n}_{dc}", tag="xt")
                       for dc in range(D // 128)]
                xns = []
                for tt in range(CH // 128):
                    xn = xload.tile([128, D], F32, name=f"xn{n}_{tt}", tag="xn")
                    nc.sync.dma_start(out=xn, in_=x_d[tcol + 128 * tt: tcol + 128 * (tt + 1), :])
                    xns.append(xn)
                for dc in range(D // 128):
                    # pack the 4 token-block transposes of one d-block into one
                    # psum bank, one copy out
                    xp = ps_tp.tile([128, CH], F32, name=f"xp{n}{dc}", tag="tp")
                    for tt in range(CH // 128):
                        nc.tensor.transpose(xp[:, 128 * tt:128 * (tt + 1)],
                                            xns[tt][:, 128 * dc:128 * (dc + 1)], ident)
                    nc.any.tensor_copy(xts[dc][:, :], xp[:, :])

                cos_c = chunks.tile([128, CH], F32, name=f"cos{n}", tag="cosc")
                sin_c = chunks.tile([128, CH], F32, name=f"sin{n}", tag="sinc")
                nc.sync.dma_start(out=cos_c, in_=cos_d[:, tcol:tcol + CH])
                nc.sync.dma_start(out=sin_c, in_=sin_d[:, tcol:tcol + CH])

                for name, wt, dst in (("q", wq, qtr), ("k", wk, ktr), ("v", wv, None)):
                    pp = ps_proj.tile([128, CH], F32, name=f"{name}ps{n}", tag="proj")
                    for dc in range(D // 128):
                        nc.tensor.matmul(pp[:, :], wt[:, dc, :], xts[dc][:, :],
                                         start=(dc == 0), stop=(dc == D // 128 - 1))
                    if name == "v":
                        # per-head natural V via PE transpose; ones row becomes
                        # the denominator column after transpose
                        for h in range(HPC):
                            vth = chunks.tile([HD + 1, CH], F32, name=f"vt{n}{h}", tag="vth")
                            nc.scalar.activation(vth[0:HD, :], pp[HD * h:HD * (h + 1), :],
                                                 AF.Identity, bias=(bv0 if h == 0 else bv1))
                            nc.vector.memset(vth[HD:HD + 1, :], 1.0)
                            for ktl in range(CH // 128):
                                vp = ps_tp.tile([128, HD + 1], F32, name=f"vp{n}{h}{ktl}", tag="tp")
                                nc.tensor.transpose(vp[:, :], vth[:, 128 * ktl:128 * (ktl + 1)],
                                                    ident[0:HD + 1, 0:HD + 1])
                                kt_glob = (tcol + 128 * ktl) // 128
                                nc.any.tensor_copy(
                                    vnat[kt_glob][:, 65 * h:65 * h + HD + 1], vp[:, :])
                    else:
                        # bias + rope: dst_chunk = (p+b)*cos + swap(p+b)*sin_signed
                        qb = chunks.tile([128, CH], F32R, name=f"{name}b{n}", tag="qb")
                        nc.scalar.activation(qb[:, :], pp[:, :], AF.Identity,
                                             bias=(bq if name == "q" else bk))
                        sw = ps_tp.tile([128, CH], F32, name=f"{name}sw{n}", tag="tp")
                        for j in range(CH // 512):
                            nc.tensor.matmul(sw[:, 512 * j:512 * (j + 1)], psw,
                                             qb[:, 512 * j:512 * (j + 1)],
                                             start=True, stop=True)
                        t1 = chunks.tile([128, CH], F32, name=f"{name}t1{n}", tag="t1")
                        t2 = chunks.tile([128, CH], F32, name=f"{name}t2{n}", tag="t2")
                        nc.vector.tensor_mul(t1[:, :], qb[:, :], cos_c[:, :])
                        nc.vector.tensor_mul(t2[:, :], sw[:, :], sin_c[:, :])
                        nc.vector.tensor_add(dst[:, tcol:tcol + CH], t1[:, :], t2[:, :])

            # ── phase 3: attention for one (batch, head, q-chunk) ───────
            def emit_att(b, h, qc):
                toff = b * S
                po = HD * h
                qcols = slice(toff + QW * qc, toff + QW * (qc + 1))
                ot = ps_ot.tile([HD + 1, QW], F32, name=f"ot{b}{h}{qc}", tag="ot")
                for kt in range(KT):
                    krows = slice(toff + 128 * kt, toff + 128 * (kt + 1))
                    sc = ps_sc.tile([128, QW], F32, name=f"sc{b}{h}{qc}{kt}", tag="sc")
                    nc.tensor.matmul(sc[:, :], ktr[po:po + HD, krows],
                                     qtr[po:po + HD, qcols], start=True, stop=True)
                    if use_mask:
                        mtile = expp.tile([128, QW], F32, name=f"mt{b}{h}{qc}{kt}", tag="mt")
                        nc.sync.dma_start(
                            out=mtile,
                            in_=mt_d[128 * kt:128 * (kt + 1), QW * qc:QW * (qc + 1)])
                        nc.vector.tensor_scalar_mul(sc[:, :], sc[:, :], SCALE)
                        nc.vector.tensor_add(sc[:, :], sc[:, :], mtile[:, :])
                    ex = expp.tile([128, QW], BF16, name=f"ex{b}{h}{qc}{kt}", tag="ex")
                    nc.scalar.activation(ex[:, :], sc[:, :], AF.Exp,
                                         scale=(1.0 if use_mask else SCALE))
                    kt_glob = (toff + 128 * kt) // 128
                    nc.tensor.matmul(ot[:, :], vnat[kt_glob][:, 65 * h:65 * h + HD + 1],
                                     ex[:, :], start=(kt == 0), stop=(kt == KT - 1))
                # normalize: rows 0..63 divided by row 64
                rec = outp.tile([1, QW], F32, name=f"rec{b}{h}{qc}", tag="rec")
                nc.vector.reciprocal(rec[:, :], ot[HD:HD + 1, :])
                bcast = outp.tile([HD, QW], F32, name=f"bc{b}{h}{qc}", tag="bc")
                nc.gpsimd.partition_broadcast(bcast[:, :], rec[:, :])
                nc.vector.tensor_mul(aoT[po:po + HD, qcols], ot[0:HD, :], bcast[:, :])

            # ── phase 4: partial output projection for one (batch,qchunk)
            def emit_oproj(b, qc):
                toff = b * S
                qcols = slice(toff + QW * qc, toff + QW * (qc + 1))
                for mo in range(D // 128):
                    yp = ps_proj.tile([128, QW], F32, name=f"yp{b}{qc}{mo}", tag="proj")
                    nc.tensor.matmul(yp[:, :], wo[:, mo, :], aoT[:, qcols],
                                     start=True, stop=True)
                    yo = outp.tile([128, QW], F32, name=f"yo{b}{qc}{mo}", tag="yo")
                    nc.any.tensor_copy(yo[:, :], yp[:, :])
                    nc.sync.dma_start(out=ypT_d[128 * mo:128 * (mo + 1), qcols], in_=yo)

            # ── emission order ──────────────────────────────────────────
            import os as _os
            if _os.environ.get("KORDER", "interleave") == "seq":
                for n in range(NCHUNK):
                    emit_chunk(n)
                for b in range(B):
                    for h in range(HPC):
                        for qc in range(QC):
                            emit_att(b, h, qc)
                    for qc in range(QC):
                        emit_oproj(b, qc)
            else:
                for n in range(4):
                    emit_chunk(n)
                for qc in range(QC):
                    emit_att(0, 0, qc)
                    emit_chunk(4 + qc)
                for qc in range(QC):
                    emit_att(0, 1, qc)
                    emit_oproj(0, qc)
                for qc in range(QC):
                    emit_att(1, 0, qc)
                for qc in range(QC):
                    emit_att(1, 1, qc)
                    emit_oproj(1, qc)

    nc.compile()
    nc.m = get_hw_module(nc.m)
    return nc


def _get_nc(use_mask: bool):
    key = ("nc", use_mask)
    if key not in _CACHE:
        _CACHE[key] = _build(use_mask)
    return _CACHE[key]


def kernel(x, rope, mask, Wq, bq, Wk, bk, Wv, bv, Wo, bo, _trace=False):
    x = np.ascontiguousarray(np.asarray(x, dtype=np.float32))
    rope = np.asarray(rope, dtype=np.float32)
    mask = np.asarray(mask, dtype=np.float32)
    use_mask = bool(np.any(mask))

    x2d = x.reshape(T, D)
    cos = rope[0, 0, :, 0, :]                      # [S, 64]
    sin = rope[1, 0, :, 0, :]
    sgn = np.where(np.arange(HD) % 2 == 0, -1.0, 1.0).astype(np.float32)[:, None]
    cosT = np.ascontiguousarray(np.tile(cos.T, (HPC, B)))          # [128, T]
    sinT = np.ascontiguousarray(np.tile(sin.T * sgn, (HPC, B)))    # [128, T]
    psw = np.zeros((128, 128), dtype=np.float32)
    idx = np.arange(128)
    psw[idx ^ 1, idx] = 1.0

    nc = _get_nc(use_mask)
    in_maps = []
    for c in range(NC):
        cs = slice(CW * c, CW * (c + 1))
        m = dict(
            x=x2d,
            wq=np.ascontiguousarray(Wq[:, cs]), bq=np.ascontiguousarray(bq[cs]).reshape(CW, 1),
            wk=np.ascontiguousarray(Wk[:, cs]), bk=np.ascontiguousarray(bk[cs]).reshape(CW, 1),
            wv=np.ascontiguousarray(Wv[:, cs]), bv=np.ascontiguousarray(bv[cs]).reshape(CW, 1),
            wo=np.ascontiguousarray(Wo[cs, :]),
            cost=cosT, sint=sinT, pswap=psw,
        )
        if use_mask:
            m["maskt"] = np.ascontiguousarray(mask[0, 0].T)
        in_maps.append({k: np.asarray(v, dtype=np.float32) for k, v in m.items()})

    res = bass_utils.run_bass_kernel_spmd(
        nc, in_maps, core_ids=list(range(NC)), trace=_trace)
    # row-parallel unshard: sum the per-core partial projections, add bias
    ypT = res.results[0]["ypT"].astype(np.float32)
    for c in range(1, NC):
        ypT = ypT + res.results[c]["ypT"]
    out = (ypT.T + np.asarray(bo, dtype=np.float32)).reshape(B, S, D).astype(np.float32)
    out = np.ascontiguousarray(out)
    if _trace:
        return out, res
    return out
